# revision 38
# baseline (speedup 1.0000x reference)
import sys

sys.path.insert(0, "/opt/trn_rl_repo")

import numpy as np
import ml_dtypes
import concourse.bass as bass  # noqa: F401  (registers types)
from concourse import bacc
import concourse.mybir as mybir
from concourse.tile import TileContext
from concourse.bass_utils import run_bass_kernel_spmd

S = 4096          # sequence length
D = 1024          # model/key/value dim
NCORES = 8
R = S // NCORES   # 512 rows per core
KK = 4            # 256-deep contraction double-chunks (DoubleRow)
J = S // 128      # 32 key tiles
VA = D + 2        # V augmented with ones column (denominator) + zero pad
CH = VA // 3      # 342-wide PV output chunks (3 chunks, one PSUM bank each)
WSC = np.float32(16.0)   # fp8 prescale for projection weights
# true scores width for key tile j under 16-row strip interleaving: strips <= j
W_TRUE = [min(R, 16 * (j + 1)) for j in range(J)]

F32 = mybir.dt.float32
BF16 = mybir.dt.bfloat16
E4 = mybir.dt.float8e4
DR = mybir.MatmulPerfMode.DoubleRow
E4NP = ml_dtypes.float8_e4m3fn

_cache = {}


def _warmup(nc, pool, psum_pool, tag, n):
    """Keep the PE continuously busy until real data arrives: the clock
    pstate ramps only after ~3us of uninterrupted execution and resets on
    idle, so the warmup must run right up to the first real matmul."""
    if n == 0:
        return
    wz = pool.tile([128, 2, 128], E4, name="wz")
    nc.vector.memset(wz[:], 0)
    wps = psum_pool.tile([128, 512], F32, name="wps", tag=tag)
    for _ in range(n):
        nc.tensor.matmul(wps[0:64, 0:128], wz[:, :, 0:64], wz[:], start=True,
                         stop=True, perf_mode=DR)


def _build_phase1(warm=24):
    """Per core: q/k/v = xs @ (16*W) for its 512-row x slice, fp8 DoubleRow.

    Three per-projection passes of 8 PSUM tiles (4 row-chunks x 2 halves) so
    each pass only waits on its own 1MB weight stream. Outputs bf16, 16x the
    true projection; host rescales and adds biases. First input chunk is tiny
    so matmuls start early; outputs stream per 128-row chunk.
      xsT [128, 4, 2, 512]: [p, kk, t, r] = x[r, 128*(2kk+t)+p]
      w*  [128, 4, 2, 1024]: [p, kk, t, f] = 16*W[128*(2kk+t)+p, f]
      out [128, 4, 1024]: [p, i, f] = proj[i*128+p, f]
    """
    nc = bacc.Bacc(None, target_bir_lowering=False)
    pk_in = nc.dram_tensor("pk", [128, 56, 512], E4, kind="ExternalInput")
    outs = [nc.dram_tensor(n, [128, 4, D], BF16, kind="ExternalOutput")
            for n in ("q", "k", "v")]
    with TileContext(nc) as tc:
        with tc.tile_pool(name="inp", bufs=1) as inp, \
             tc.tile_pool(name="ps", bufs=8, space="PSUM") as ps:
            pk = inp.tile([128, 56, 512], E4)
            for a, b in ((0, 2), (2, 4), (4, 6), (6, 12), (12, 18), (18, 24),
                         (24, 32), (32, 40), (40, 56)):
                nc.sync.dma_start(pk[:, a:b], pk_in[:, a:b])
            _warmup(nc, inp, ps, "ps", warm)

            def rhs_ap(w_i, kk, h):
                if w_i == 0:
                    return pk[:, 6 * kk + 2 + 2 * h : 6 * kk + 4 + 2 * h]
                c0 = 8 + 16 * w_i + 4 * kk + 2 * h
                return pk[:, c0 : c0 + 2]

            # projections 0/1: kk-outer (follows the weight DMA stream);
            # their evacs hide under the next projection's matmuls
            for w_i in range(2):
                pz = [ps.tile([128, 512], F32, name=f"p{w_i}_{n2}", tag="ps")
                      for n2 in range(8)]
                for kk in range(KK):
                    for i in range(R // 128):
                        lhsT = pk[:, 6 * kk : 6 * kk + 2, i * 128 : (i + 1) * 128]
                        for h in range(2):
                            nc.tensor.matmul(
                                pz[i * 2 + h][:],
                                lhsT,
                                rhs_ap(w_i, kk, h),
                                start=(kk == 0), stop=(kk == KK - 1),
                                perf_mode=DR,
                            )
                osb = inp.tile([128, 4, D], BF16, name=f"o{w_i}")
                for i in range(R // 128):
                    for h in range(2):
                        dst = osb[:, i, h * 512 : (h + 1) * 512]
                        if h == 0:
                            nc.vector.tensor_copy(dst, pz[i * 2 + h][:])
                        else:
                            nc.scalar.copy(dst, pz[i * 2 + h][:])
                # one 8KB-per-partition-line DMA; drains under later matmuls
                nc.sync.dma_start(outs[w_i][:], osb[:])
            # last projection: i-outer so each 128-row chunk evacuates and
            # streams out while later chunks are still on the PE (weights for
            # it have fully arrived by now, so stream order doesn't matter)
            osb = inp.tile([128, 4, D], BF16, name="o2")
            for i in range(R // 128):
                pz = [ps.tile([128, 512], F32, name=f"p2_{i}_{h}", tag="ps")
                      for h in range(2)]
                for kk in range(KK):
                    lhsT = pk[:, 6 * kk : 6 * kk + 2, i * 128 : (i + 1) * 128]
                    for h in range(2):
                        nc.tensor.matmul(
                            pz[h][:],
                            lhsT,
                            rhs_ap(2, kk, h),
                            start=(kk == 0), stop=(kk == KK - 1),
                            perf_mode=DR,
                        )
                nc.vector.tensor_copy(osb[:, i, 0:512], pz[0][:])
                nc.scalar.copy(osb[:, i, 512:1024], pz[1][:])
                if i == 1:
                    nc.sync.dma_start(outs[2][:, 0:2], osb[:, 0:2])
                elif i >= 2:
                    nc.sync.dma_start(outs[2][:, i], osb[:, i])
    nc.finalize()
    return nc


def _build_phase2(warm=34):
    """Per core: anti-causal attention for its 512 query rows (16-row strips
    s = 8k+c, k=0..31) vs all 4096 keys; exact-triangle sparsity with a
    uniform SPMD program (all per-core variation lives in input data).

    Scores and PV are interleaved in four blocks (key tiles 31..24 then PV for
    query chunk 3, etc.) so the PE, scalar (exp) and vector (mask) pipelines
    overlap instead of running as two serial phases. Each score tile j splits
    into a clean region [0,16j) where every key sees every query (exp written
    to fp8 directly on scalar, no mask op) and a 16-wide partial region where
    the diagonal mask applies (small exp + masked multiply on vector). The
    16-wide dead strips of even tiles are zeroed once up front so PV key-pair
    reads stay in defined territory.
      qt [128, 4, 2, 512]: [p,kk,t,q] = qT[128*(2kk+t)+p, strip-ordered q]
      kt [128, 32, 4, 2, 128]: [p,j,kk,t,c] = kT[128*(2kk+t)+p, 128j+c]
      vi [128, 32, 1026]: [p,j,c] = v_aug[128j+p, c]
      th [128, 32]: #queries visible to key 128j+p, minus one
      rd [128, 4, 1026] bf16: [p,qc,:] = unnormalized read for query col
      qc*128+p (strip order) + denominator column.
    """
    nc = bacc.Bacc(None, target_bir_lowering=False)
    qt_in = nc.dram_tensor("qt", [128, KK, 2, R], E4, kind="ExternalInput")
    kt_in = nc.dram_tensor("kt", [128, J, KK, 2, 128], E4, kind="ExternalInput")
    v_in = nc.dram_tensor("vi", [128, J, VA], E4, kind="ExternalInput")
    thr = nc.dram_tensor("th", [128, J], F32, kind="ExternalInput")
    rd = nc.dram_tensor("rd", [128, 4, VA], BF16, kind="ExternalOutput")
    with TileContext(nc) as tc:
        with tc.tile_pool(name="cst", bufs=1) as cst, \
             tc.tile_pool(name="pp", bufs=8, space="PSUM") as pp:
            qt = cst.tile([128, KK, 2, R], E4)
            kt = cst.tile([128, J, KK, 2, 128], E4)
            vt = cst.tile([128, J, VA], E4)
            th = cst.tile([128, J], F32)
            nc.scalar.dma_start(th[:], thr[:])
            # single sync HWDGE stream at ~330GB/s, ordered just-in-time
            # against the B3 B2 (P2|B1) (P1|B0) P0 P3 schedule below
            nc.sync.dma_start(qt[:], qt_in[:])
            nc.sync.dma_start(kt[:, 31:32], kt_in[:, 31:32])
            nc.sync.dma_start(kt[:, 28:31], kt_in[:, 28:31])
            nc.sync.dma_start(kt[:, 24:28], kt_in[:, 24:28])
            nc.sync.dma_start(kt[:, 16:24], kt_in[:, 16:24])
            nc.sync.dma_start(vt[:, 24:32], v_in[:, 24:32])
            nc.sync.dma_start(kt[:, 12:16], kt_in[:, 12:16])
            nc.sync.dma_start(vt[:, 16:24], v_in[:, 16:24])
            nc.sync.dma_start(kt[:, 8:12], kt_in[:, 8:12])
            nc.sync.dma_start(kt[:, 4:8], kt_in[:, 4:8])
            nc.sync.dma_start(kt[:, 0:4], kt_in[:, 0:4])
            nc.sync.dma_start(vt[:, 8:16], v_in[:, 8:16])
            nc.sync.dma_start(vt[:, 0:8], v_in[:, 0:8])
            _warmup(nc, cst, pp, "p", warm)
            io = cst.tile([128, R], F32)
            nc.gpsimd.iota(io[:], [[1, R]], channel_multiplier=0,
                           allow_small_or_imprecise_dtypes=True)
            pm = [cst.tile([128, 2, R], E4, name=f"pm{m}") for m in range(J // 2)]
            # static dead strips of even tiles: cols [16*(2m+1), 16*(2m+2))
            for m in range(J // 2):
                z0, z1 = W_TRUE[2 * m], min(R, 16 * (2 * m + 2))
                if z1 > z0:
                    nc.gpsimd.memset(pm[m][:, 0, z0:z1], 0)
            o4 = cst.tile([128, 4, VA], BF16)

            exp_scale = float(1.0 / np.sqrt(D))
            exs = [cst.tile([128, 512], F32, name=f"ex{i}") for i in range(6)]
            exn = [0]

            def pack_thunks(js):
                """Thunks for one scores-psum generation covering several key
                tiles at disjoint column ranges (packing small tiles cuts the
                PE<->scalar psum-rotation latency chain), ending with the
                exp + mask finisher. All psum tiles come from the single
                8-bank pool so score packs get deep rotation lookahead."""
                js = tuple(js)
                state = {}
                thunks = []
                off = 0
                for ji, j in enumerate(js):
                    w = W_TRUE[j]
                    for kk in range(KK):
                        def t(j=j, kk=kk, o=off, w=w, alloc=(ji == 0 and kk == 0)):
                            if alloc:
                                state["ps"] = pp.tile([128, 512], F32,
                                                      name=f"s{js[0]}", tag="p")
                            nc.tensor.matmul(
                                state["ps"][:, o : o + w],
                                kt[:, j, kk],
                                qt[:, kk, :, 0:w],
                                start=(kk == 0), stop=(kk == KK - 1),
                                perf_mode=DR,
                                skip_group_check=True,
                            )
                        thunks.append(t)
                    off += w

                def fin(total=off):
                    ps_ = state["ps"]
                    ex = exs[exn[0] % 6]
                    exn[0] += 1
                    nc.scalar.activation(ex[:, 0:total], ps_[:, 0:total],
                                         mybir.ActivationFunctionType.Exp,
                                         scale=exp_scale)
                    o2 = 0
                    for j in js:
                        w = W_TRUE[j]
                        nc.vector.scalar_tensor_tensor(
                            pm[j // 2][:, j % 2, 0:w], io[:, 0:w],
                            th[:, j : j + 1], ex[:, o2 : o2 + w],
                            op0=mybir.AluOpType.is_le, op1=mybir.AluOpType.mult,
                        )
                        o2 += w
                thunks.append(fin)
                return thunks

            def pv_thunks(qc):
                state = {}
                ms = list(range(J // 2 - 1, 4 * qc - 1, -1))
                thunks = []
                for mi, m in enumerate(ms):
                    wd = min(128, 16 * (2 * m + 2) - 128 * qc)
                    for ch in range(3):
                        def t(m=m, ch=ch, wd=wd, alloc=(mi == 0 and ch == 0),
                              first=(mi == 0), last=(m == ms[-1])):
                            if alloc:
                                state["pz"] = [
                                    pp.tile([128, 512], F32, name=f"pv{qc}_{c}",
                                            tag="p")
                                    for c in range(3)]
                            nc.tensor.matmul(
                                state["pz"][ch][0:wd, 0:CH],
                                pm[m][:, :, qc * 128 : qc * 128 + wd],
                                vt[:, 2 * m : 2 * m + 2, ch * CH : (ch + 1) * CH],
                                start=first, stop=last,
                                perf_mode=DR,
                                skip_group_check=True,
                            )
                        thunks.append(t)

                def fin():
                    pz = state["pz"]
                    sl = SLOT[qc]
                    nc.vector.tensor_copy(o4[:, sl, 0:CH], pz[0][:, 0:CH])
                    nc.scalar.copy(o4[:, sl, CH : 2 * CH], pz[1][:, 0:CH])
                    nc.vector.tensor_copy(o4[:, sl, 2 * CH : VA], pz[2][:, 0:CH])
                thunks.append(fin)
                return thunks

            def weave(a, b, lead=0):
                # proportional merge preserving each stream's order; `lead`
                # primary thunks run before the secondary starts (so late
                # stream chunks arrive before their consumers)
                na, nb = len(a), len(b)
                ia = ib = 0
                while ia < na or ib < nb:
                    if ib >= nb or (ia < na and (ia < lead or
                                    (ia - lead) * nb <= ib * (na - lead))):
                        a[ia](); ia += 1
                    else:
                        b[ib](); ib += 1

            # tile packs per scores-psum generation: singles while w > 256,
            # pairs for j 15..8, quads for j 7..0. Small-tile scores weave
            # between PV matmuls so their LDWEIGHTS hide under PV's 342-wide
            # streams; P3 (smallest PV) goes last so the final output DMA
            # gates on 1.7us of PE work instead of P0's 6.9us.
            SLOT = {2: 0, 1: 1, 0: 2, 3: 3}  # qc -> output slot, by finish time
            B3 = [[31], [30], [29], [28], [27], [26], [25], [24]]
            B2 = [[23], [22], [21], [20], [19], [18], [17], [16]]
            B1 = [[15, 14], [13, 12], [11, 10], [9, 8]]
            B0 = [[7, 6, 5, 4], [3, 2, 1, 0]]
            flat = lambda blocks: [t for js in blocks for t in pack_thunks(js)]
            for t in flat(B3) + flat(B2):
                t()
            weave(pv_thunks(2), flat(B1))
            weave(pv_thunks(1), flat(B0))
            nc.sync.dma_start(rd[:, 0:2], o4[:, 0:2])   # slots qc2, qc1
            weave(pv_thunks(0), pv_thunks(3))
            nc.sync.dma_start(rd[:, 2:4], o4[:, 2:4])   # slots qc0, qc3

    nc.finalize()
    return nc


def _dr_layout(aT):
    # [1024, C] (contraction-major) -> [128, 4, 2, C] DoubleRow layout
    c = aT.shape[1]
    return np.ascontiguousarray(aT.reshape(KK, 2, 128, c).transpose(2, 0, 1, 3))


def _strip_rows(c):
    # query rows owned by core c in on-device column order (16-row strips)
    q = np.arange(R)
    return 128 * (q // 16) + 16 * c + (q % 16)


def prep_phase1(x, Wq, Wk, Wv):
    xq = x.astype(E4NP)
    w_ins = [np.ascontiguousarray(
        _dr_layout((W * WSC).astype(E4NP).reshape(D, D))) for W in (Wq, Wk, Wv)]
    in_maps = []
    for c in range(NCORES):
        xsT = _dr_layout(np.ascontiguousarray(xq[c * R : (c + 1) * R].T))
        pk = np.zeros((128, 56, 512), E4NP)
        for kk in range(KK):
            for t in range(2):
                pk[:, 6 * kk + t] = xsT[:, kk, t]
                for h in range(2):
                    pk[:, 6 * kk + 2 + 2 * h + t] = w_ins[0][:, kk, t, h * 512 : (h + 1) * 512]
                    for w_i in (1, 2):
                        pk[:, 8 + 16 * w_i + 4 * kk + 2 * h + t] = \
                            w_ins[w_i][:, kk, t, h * 512 : (h + 1) * 512]
        in_maps.append({"pk": pk})
    return in_maps


def _p1out(res, name):
    # [128, 4, D] -> [512, D] float32
    return res[name].astype(np.float32).transpose(1, 0, 2).reshape(R, D)


def prep_phase2(res1, bq, bk, bv):
    inv = np.float32(1.0 / WSC)
    q_g = np.concatenate([_p1out(res1[c], "q") for c in range(NCORES)]) * inv + bq
    k_g = np.concatenate([_p1out(res1[c], "k") for c in range(NCORES)]) * inv + bk
    v_g = np.concatenate([_p1out(res1[c], "v") for c in range(NCORES)]) * inv + bv
    kT = np.ascontiguousarray(k_g.T.astype(E4NP))
    kt_in = np.ascontiguousarray(
        kT.reshape(KK, 2, 128, J, 128).transpose(2, 3, 0, 1, 4))
    v_aug = np.concatenate(
        [v_g, np.ones((S, 1), np.float32), np.zeros((S, 1), np.float32)], axis=1)
    v_in = np.ascontiguousarray(
        v_aug.astype(E4NP).reshape(J, 128, VA).transpose(1, 0, 2))
    qT8 = q_g.T.astype(E4NP)
    p_idx = np.arange(128)[:, None]
    j_idx = np.arange(J)[None, :]
    keys = (128 * j_idx + p_idx).ravel()
    in_maps = []
    for c in range(NCORES):
        rows = _strip_rows(c)
        qt = _dr_layout(np.ascontiguousarray(qT8[:, rows]))
        th_c = (np.searchsorted(rows, keys, side="right") - 1).reshape(128, J)
        in_maps.append({"qt": qt, "kt": kt_in, "vi": v_in,
                        "th": np.ascontiguousarray(th_c.astype(np.float32))})
    return in_maps


def finish(x, res2):
    read = np.empty((S, D), np.float32)
    for c in range(NCORES):
        r = res2[c]["rd"][:, [2, 1, 0, 3]].astype(np.float32)
        r = r.transpose(1, 0, 2).reshape(R, VA)
        read[_strip_rows(c)] = r[:, :D] / r[:, D : D + 1]
    return np.concatenate([x, read], axis=1)


def kernel(x, Wk, bk, Wq, bq, Wv, bv):
    x = np.asarray(x, dtype=np.float32)
    Wk = np.asarray(Wk, dtype=np.float32)
    Wq = np.asarray(Wq, dtype=np.float32)
    Wv = np.asarray(Wv, dtype=np.float32)
    bk = np.asarray(bk, dtype=np.float32)
    bq = np.asarray(bq, dtype=np.float32)
    bv = np.asarray(bv, dtype=np.float32)

    if "p1" not in _cache:
        _cache["p1"] = _build_phase1()
    if "p2" not in _cache:
        _cache["p2"] = _build_phase2()

    in_maps1 = prep_phase1(x, Wq, Wk, Wv)
    res1 = run_bass_kernel_spmd(_cache["p1"], in_maps1, list(range(NCORES))).results
    in_maps2 = prep_phase2(res1, bq, bk, bv)
    res2 = run_bass_kernel_spmd(_cache["p2"], in_maps2, list(range(NCORES))).results
    return finish(x, res2)


# revision 39
# speedup vs baseline: 1.0246x; 1.0246x over previous
import sys

sys.path.insert(0, "/opt/trn_rl_repo")

import numpy as np
import ml_dtypes
import concourse.bass as bass  # noqa: F401  (registers types)
from concourse import bacc
import concourse.mybir as mybir
from concourse.tile import TileContext
from concourse.bass_utils import run_bass_kernel_spmd

S = 4096          # sequence length
D = 1024          # model/key/value dim
NCORES = 8
R = S // NCORES   # 512 rows per core
KK = 4            # 256-deep contraction double-chunks (DoubleRow)
J = S // 128      # 32 key tiles
VA = D + 2        # V augmented with ones column (denominator) + zero pad
CH = VA // 3      # 342-wide PV output chunks (3 chunks, one PSUM bank each)
WSC = np.float32(16.0)   # fp8 prescale for projection weights
# true scores width for key tile j under 16-row strip interleaving: strips <= j
W_TRUE = [min(R, 16 * (j + 1)) for j in range(J)]

F32 = mybir.dt.float32
BF16 = mybir.dt.bfloat16
E4 = mybir.dt.float8e4
DR = mybir.MatmulPerfMode.DoubleRow
E4NP = ml_dtypes.float8_e4m3fn

_cache = {}


def _warmup(nc, pool, psum_pool, tag, n):
    """Keep the PE continuously busy until real data arrives: the clock
    pstate ramps only after ~3us of uninterrupted execution and resets on
    idle, so the warmup must run right up to the first real matmul."""
    if n == 0:
        return
    wz = pool.tile([128, 2, 128], E4, name="wz")
    nc.vector.memset(wz[:], 0)
    wps = psum_pool.tile([128, 512], F32, name="wps", tag=tag)
    for _ in range(n):
        nc.tensor.matmul(wps[0:64, 0:128], wz[:, :, 0:64], wz[:], start=True,
                         stop=True, perf_mode=DR)


def _build_phase1(warm=24):
    """Per core: q/k/v = xs @ (16*W) for its 512-row x slice, fp8 DoubleRow.

    Three per-projection passes of 8 PSUM tiles (4 row-chunks x 2 halves) so
    each pass only waits on its own 1MB weight stream. Outputs bf16, 16x the
    true projection; host rescales and adds biases. First input chunk is tiny
    so matmuls start early; outputs stream per 128-row chunk.
      xsT [128, 4, 2, 512]: [p, kk, t, r] = x[r, 128*(2kk+t)+p]
      w*  [128, 4, 2, 1024]: [p, kk, t, f] = 16*W[128*(2kk+t)+p, f]
      out [128, 4, 1024]: [p, i, f] = proj[i*128+p, f]
    """
    nc = bacc.Bacc(None, target_bir_lowering=False)
    pk_in = nc.dram_tensor("pk", [128, 56, 512], E4, kind="ExternalInput")
    outs = [nc.dram_tensor(n, [128, 4, D], BF16, kind="ExternalOutput")
            for n in ("q", "k", "v")]
    with TileContext(nc) as tc:
        with tc.tile_pool(name="inp", bufs=1) as inp, \
             tc.tile_pool(name="ps", bufs=8, space="PSUM") as ps:
            pk = inp.tile([128, 56, 512], E4)
            for a, b in ((0, 2), (2, 4), (4, 6), (6, 12), (12, 18), (18, 24),
                         (24, 32), (32, 40), (40, 56)):
                nc.sync.dma_start(pk[:, a:b], pk_in[:, a:b])
            _warmup(nc, inp, ps, "ps", warm)

            def rhs_ap(w_i, kk, h):
                if w_i == 0:
                    return pk[:, 6 * kk + 2 + 2 * h : 6 * kk + 4 + 2 * h]
                c0 = 8 + 16 * w_i + 4 * kk + 2 * h
                return pk[:, c0 : c0 + 2]

            # projections 0/1: kk-outer (follows the weight DMA stream);
            # their evacs hide under the next projection's matmuls
            for w_i in range(2):
                pz = [ps.tile([128, 512], F32, name=f"p{w_i}_{n2}", tag="ps")
                      for n2 in range(8)]
                for kk in range(KK):
                    for i in range(R // 128):
                        lhsT = pk[:, 6 * kk : 6 * kk + 2, i * 128 : (i + 1) * 128]
                        for h in range(2):
                            nc.tensor.matmul(
                                pz[i * 2 + h][:],
                                lhsT,
                                rhs_ap(w_i, kk, h),
                                start=(kk == 0), stop=(kk == KK - 1),
                                perf_mode=DR,
                            )
                osb = inp.tile([128, 4, D], BF16, name=f"o{w_i}")
                for i in range(R // 128):
                    for h in range(2):
                        dst = osb[:, i, h * 512 : (h + 1) * 512]
                        if h == 0:
                            nc.vector.tensor_copy(dst, pz[i * 2 + h][:])
                        else:
                            nc.scalar.copy(dst, pz[i * 2 + h][:])
                # one 8KB-per-partition-line DMA; drains under later matmuls
                nc.sync.dma_start(outs[w_i][:], osb[:])
            # last projection: i-outer so each 128-row chunk evacuates and
            # streams out while later chunks are still on the PE (weights for
            # it have fully arrived by now, so stream order doesn't matter)
            osb = inp.tile([128, 4, D], BF16, name="o2")
            for i in range(R // 128):
                pz = [ps.tile([128, 512], F32, name=f"p2_{i}_{h}", tag="ps")
                      for h in range(2)]
                for kk in range(KK):
                    lhsT = pk[:, 6 * kk : 6 * kk + 2, i * 128 : (i + 1) * 128]
                    for h in range(2):
                        nc.tensor.matmul(
                            pz[h][:],
                            lhsT,
                            rhs_ap(2, kk, h),
                            start=(kk == 0), stop=(kk == KK - 1),
                            perf_mode=DR,
                        )
                nc.vector.tensor_copy(osb[:, i, 0:512], pz[0][:])
                nc.scalar.copy(osb[:, i, 512:1024], pz[1][:])
                if i == 1:
                    nc.sync.dma_start(outs[2][:, 0:2], osb[:, 0:2])
                elif i >= 2:
                    nc.sync.dma_start(outs[2][:, i], osb[:, i])
    nc.finalize()
    return nc


def _build_phase2(warm=34):
    """Per core: anti-causal attention for its 512 query rows (16-row strips
    s = 8k+c, k=0..31) vs all 4096 keys; exact-triangle sparsity with a
    uniform SPMD program (all per-core variation lives in input data).

    Scores and PV are interleaved in four blocks (key tiles 31..24 then PV for
    query chunk 3, etc.) so the PE, scalar (exp) and vector (mask) pipelines
    overlap instead of running as two serial phases. Each score tile j splits
    into a clean region [0,16j) where every key sees every query (exp written
    to fp8 directly on scalar, no mask op) and a 16-wide partial region where
    the diagonal mask applies (small exp + masked multiply on vector). The
    16-wide dead strips of even tiles are zeroed once up front so PV key-pair
    reads stay in defined territory.
      qt [128, 4, 2, 512]: [p,kk,t,q] = qT[128*(2kk+t)+p, strip-ordered q]
      kt [128, 32, 4, 2, 128]: [p,j,kk,t,c] = kT[128*(2kk+t)+p, 128j+c]
      vi [128, 32, 1026]: [p,j,c] = v_aug[128j+p, c]
      th [128, 32]: #queries visible to key 128j+p, minus one
      rd [128, 4, 1026] bf16: [p,qc,:] = unnormalized read for query col
      qc*128+p (strip order) + denominator column.
    """
    nc = bacc.Bacc(None, target_bir_lowering=False)
    qt_in = nc.dram_tensor("qt", [128, KK, 2, R], E4, kind="ExternalInput")
    kt_in = nc.dram_tensor("kt", [128, J, KK, 2, 128], E4, kind="ExternalInput")
    v_in = nc.dram_tensor("vi", [128, J, VA], E4, kind="ExternalInput")
    thr = nc.dram_tensor("th", [128, J], F32, kind="ExternalInput")
    rd = nc.dram_tensor("rd", [128, 4, VA], BF16, kind="ExternalOutput")
    with TileContext(nc) as tc:
        with tc.tile_pool(name="cst", bufs=1) as cst, \
             tc.tile_pool(name="pp", bufs=8, space="PSUM") as pp:
            qt = cst.tile([128, KK, 2, R], E4)
            kt = cst.tile([128, J, KK, 2, 128], E4)
            vt = cst.tile([128, J, VA], E4)
            th = cst.tile([128, J], F32)
            nc.scalar.dma_start(th[:], thr[:])
            # single sync HWDGE stream at ~330GB/s, ordered just-in-time
            # against the B3 B2 (P2|B1) (P1|B0) P0 P3 schedule below
            nc.sync.dma_start(qt[:], qt_in[:])
            nc.sync.dma_start(kt[:, 31:32], kt_in[:, 31:32])
            nc.sync.dma_start(kt[:, 28:31], kt_in[:, 28:31])
            nc.sync.dma_start(kt[:, 24:28], kt_in[:, 24:28])
            nc.sync.dma_start(kt[:, 16:24], kt_in[:, 16:24])
            nc.sync.dma_start(vt[:, 24:32], v_in[:, 24:32])
            nc.sync.dma_start(kt[:, 12:16], kt_in[:, 12:16])
            nc.sync.dma_start(vt[:, 16:24], v_in[:, 16:24])
            nc.sync.dma_start(kt[:, 8:12], kt_in[:, 8:12])
            nc.sync.dma_start(kt[:, 0:8], kt_in[:, 0:8])
            nc.sync.dma_start(vt[:, 8:16], v_in[:, 8:16])
            nc.sync.dma_start(vt[:, 0:8], v_in[:, 0:8])
            _warmup(nc, cst, pp, "p", warm)
            io = cst.tile([128, R], F32)
            nc.gpsimd.iota(io[:], [[1, R]], channel_multiplier=0,
                           allow_small_or_imprecise_dtypes=True)
            pm = [cst.tile([128, 2, R], E4, name=f"pm{m}") for m in range(J // 2)]
            # static dead strips of even tiles: cols [16*(2m+1), 16*(2m+2))
            for m in range(J // 2):
                z0, z1 = W_TRUE[2 * m], min(R, 16 * (2 * m + 2))
                if z1 > z0:
                    nc.gpsimd.memset(pm[m][:, 0, z0:z1], 0)
            o4 = cst.tile([128, 4, VA], BF16)

            exp_scale = float(1.0 / np.sqrt(D))
            exs = [cst.tile([128, 512], F32, name=f"ex{i}") for i in range(6)]
            exn = [0]

            def pack_thunks(js):
                """Thunks for one scores-psum generation covering several key
                tiles at disjoint column ranges (packing small tiles cuts the
                PE<->scalar psum-rotation latency chain), ending with the
                exp + mask finisher. All psum tiles come from the single
                8-bank pool so score packs get deep rotation lookahead."""
                js = tuple(js)
                state = {}
                thunks = []
                off = 0
                for ji, j in enumerate(js):
                    w = W_TRUE[j]
                    for kk in range(KK):
                        def t(j=j, kk=kk, o=off, w=w, alloc=(ji == 0 and kk == 0)):
                            if alloc:
                                state["ps"] = pp.tile([128, 512], F32,
                                                      name=f"s{js[0]}", tag="p")
                            nc.tensor.matmul(
                                state["ps"][:, o : o + w],
                                kt[:, j, kk],
                                qt[:, kk, :, 0:w],
                                start=(kk == 0), stop=(kk == KK - 1),
                                perf_mode=DR,
                                skip_group_check=True,
                            )
                        thunks.append(t)
                    off += w

                def fin(total=off):
                    ps_ = state["ps"]
                    ex = exs[exn[0] % 6]
                    exn[0] += 1
                    nc.scalar.activation(ex[:, 0:total], ps_[:, 0:total],
                                         mybir.ActivationFunctionType.Exp,
                                         scale=exp_scale)
                    o2 = 0
                    for j in js:
                        w = W_TRUE[j]
                        nc.vector.scalar_tensor_tensor(
                            pm[j // 2][:, j % 2, 0:w], io[:, 0:w],
                            th[:, j : j + 1], ex[:, o2 : o2 + w],
                            op0=mybir.AluOpType.is_le, op1=mybir.AluOpType.mult,
                        )
                        o2 += w
                thunks.append(fin)
                return thunks

            def pv_thunks(qc):
                state = {}
                ms = list(range(J // 2 - 1, 4 * qc - 1, -1))
                thunks = []
                for mi, m in enumerate(ms):
                    wd = min(128, 16 * (2 * m + 2) - 128 * qc)
                    for ch in range(3):
                        def t(m=m, ch=ch, wd=wd, alloc=(mi == 0 and ch == 0),
                              first=(mi == 0), last=(m == ms[-1])):
                            if alloc:
                                state["pz"] = [
                                    pp.tile([128, 512], F32, name=f"pv{qc}_{c}",
                                            tag="p")
                                    for c in range(3)]
                            nc.tensor.matmul(
                                state["pz"][ch][0:wd, 0:CH],
                                pm[m][:, :, qc * 128 : qc * 128 + wd],
                                vt[:, 2 * m : 2 * m + 2, ch * CH : (ch + 1) * CH],
                                start=first, stop=last,
                                perf_mode=DR,
                                skip_group_check=True,
                            )
                        thunks.append(t)

                def fin():
                    pz = state["pz"]
                    sl = SLOT[qc]
                    nc.vector.tensor_copy(o4[:, sl, 0:CH], pz[0][:, 0:CH])
                    nc.scalar.copy(o4[:, sl, CH : 2 * CH], pz[1][:, 0:CH])
                    nc.vector.tensor_copy(o4[:, sl, 2 * CH : VA], pz[2][:, 0:CH])
                thunks.append(fin)
                return thunks

            def weave(a, b, lead=0):
                # proportional merge preserving each stream's order; `lead`
                # primary thunks run before the secondary starts (so late
                # stream chunks arrive before their consumers)
                na, nb = len(a), len(b)
                ia = ib = 0
                while ia < na or ib < nb:
                    if ib >= nb or (ia < na and (ia < lead or
                                    (ia - lead) * nb <= ib * (na - lead))):
                        a[ia](); ia += 1
                    else:
                        b[ib](); ib += 1

            # tile packs per scores-psum generation: singles while w > 256,
            # pairs for j 15..8, quads for j 7..0. Small-tile scores weave
            # between PV matmuls so their LDWEIGHTS hide under PV's 342-wide
            # streams; P3 (smallest PV) goes last so the final output DMA
            # gates on 1.7us of PE work instead of P0's 6.9us.
            SLOT = {2: 0, 1: 1, 0: 2, 3: 3}  # qc -> output slot, by finish time
            B3 = [[31], [30], [29], [28], [27], [26], [25], [24]]
            B2 = [[23], [22], [21], [20], [19], [18], [17], [16]]
            B1 = [[15, 14], [13, 12], [11, 10], [9, 8]]
            B0 = [[7, 6, 5, 4], [3, 2, 1, 0]]
            flat = lambda blocks: [t for js in blocks for t in pack_thunks(js)]
            for t in flat(B3) + flat(B2):
                t()
            weave(pv_thunks(2), flat(B1))
            weave(pv_thunks(1), flat(B0))
            nc.sync.dma_start(rd[:, 0:2], o4[:, 0:2])   # slots qc2, qc1
            weave(pv_thunks(0), pv_thunks(3))
            nc.sync.dma_start(rd[:, 2:4], o4[:, 2:4])   # slots qc0, qc3

    nc.finalize()
    return nc


def _dr_layout(aT):
    # [1024, C] (contraction-major) -> [128, 4, 2, C] DoubleRow layout
    c = aT.shape[1]
    return np.ascontiguousarray(aT.reshape(KK, 2, 128, c).transpose(2, 0, 1, 3))


def _strip_rows(c):
    # query rows owned by core c in on-device column order (16-row strips)
    q = np.arange(R)
    return 128 * (q // 16) + 16 * c + (q % 16)


def prep_phase1(x, Wq, Wk, Wv):
    xq = x.astype(E4NP)
    w_ins = [np.ascontiguousarray(
        _dr_layout((W * WSC).astype(E4NP).reshape(D, D))) for W in (Wq, Wk, Wv)]
    in_maps = []
    for c in range(NCORES):
        xsT = _dr_layout(np.ascontiguousarray(xq[c * R : (c + 1) * R].T))
        pk = np.zeros((128, 56, 512), E4NP)
        for kk in range(KK):
            for t in range(2):
                pk[:, 6 * kk + t] = xsT[:, kk, t]
                for h in range(2):
                    pk[:, 6 * kk + 2 + 2 * h + t] = w_ins[0][:, kk, t, h * 512 : (h + 1) * 512]
                    for w_i in (1, 2):
                        pk[:, 8 + 16 * w_i + 4 * kk + 2 * h + t] = \
                            w_ins[w_i][:, kk, t, h * 512 : (h + 1) * 512]
        in_maps.append({"pk": pk})
    return in_maps


def _p1out(res, name):
    # [128, 4, D] -> [512, D] float32
    return res[name].astype(np.float32).transpose(1, 0, 2).reshape(R, D)


def prep_phase2(res1, bq, bk, bv):
    inv = np.float32(1.0 / WSC)
    q_g = np.concatenate([_p1out(res1[c], "q") for c in range(NCORES)]) * inv + bq
    k_g = np.concatenate([_p1out(res1[c], "k") for c in range(NCORES)]) * inv + bk
    v_g = np.concatenate([_p1out(res1[c], "v") for c in range(NCORES)]) * inv + bv
    kT = np.ascontiguousarray(k_g.T.astype(E4NP))
    kt_in = np.ascontiguousarray(
        kT.reshape(KK, 2, 128, J, 128).transpose(2, 3, 0, 1, 4))
    v_aug = np.concatenate(
        [v_g, np.ones((S, 1), np.float32), np.zeros((S, 1), np.float32)], axis=1)
    v_in = np.ascontiguousarray(
        v_aug.astype(E4NP).reshape(J, 128, VA).transpose(1, 0, 2))
    qT8 = q_g.T.astype(E4NP)
    p_idx = np.arange(128)[:, None]
    j_idx = np.arange(J)[None, :]
    keys = (128 * j_idx + p_idx).ravel()
    in_maps = []
    for c in range(NCORES):
        rows = _strip_rows(c)
        qt = _dr_layout(np.ascontiguousarray(qT8[:, rows]))
        th_c = (np.searchsorted(rows, keys, side="right") - 1).reshape(128, J)
        in_maps.append({"qt": qt, "kt": kt_in, "vi": v_in,
                        "th": np.ascontiguousarray(th_c.astype(np.float32))})
    return in_maps


def finish(x, res2):
    read = np.empty((S, D), np.float32)
    for c in range(NCORES):
        r = res2[c]["rd"][:, [2, 1, 0, 3]].astype(np.float32)
        r = r.transpose(1, 0, 2).reshape(R, VA)
        read[_strip_rows(c)] = r[:, :D] / r[:, D : D + 1]
    return np.concatenate([x, read], axis=1)


def kernel(x, Wk, bk, Wq, bq, Wv, bv):
    x = np.asarray(x, dtype=np.float32)
    Wk = np.asarray(Wk, dtype=np.float32)
    Wq = np.asarray(Wq, dtype=np.float32)
    Wv = np.asarray(Wv, dtype=np.float32)
    bk = np.asarray(bk, dtype=np.float32)
    bq = np.asarray(bq, dtype=np.float32)
    bv = np.asarray(bv, dtype=np.float32)

    if "p1" not in _cache:
        _cache["p1"] = _build_phase1()
    if "p2" not in _cache:
        _cache["p2"] = _build_phase2()

    in_maps1 = prep_phase1(x, Wq, Wk, Wv)
    res1 = run_bass_kernel_spmd(_cache["p1"], in_maps1, list(range(NCORES))).results
    in_maps2 = prep_phase2(res1, bq, bk, bv)
    res2 = run_bass_kernel_spmd(_cache["p2"], in_maps2, list(range(NCORES))).results
    return finish(x, res2)
